# revision 15
# baseline (speedup 1.0000x reference)
"""GCN (PyG GCNConv) forward on 8 Trainium2 NeuronCores.

Reference computes z = D^-1/2 (A+I) D^-1/2 (X @ W2) + b2  (conv1 is dead code,
its result is never used).

Strategy ("message-GEMM", 1D destination partition):
  * Host: compute degrees + symmetric normalization, fold isd[src] into X
    (bf16), partition messages (edges + self loops) by destination shard
    (8 cores x 6250 nodes), degree-sort dst slots, and materialize the
    per-message source columns as one dense bf16 operand
    xtm[128 feats, S slots] per core (padding columns are zero).
  * Device (identical program on all 8 cores, per-core data via in_maps):
    stream xtm sequentially (full HBM rate, no gather descriptors at all);
    for each batch of g dst tiles the degree layers are accumulated directly
    in PSUM by the tensor engine:
        psum[64, g*128] (+)= W2.T @ xtm[:, layer d columns]   d = 0..db-1
    so the segmented sum over incoming messages IS the matmul accumulation.
    One DVE op per batch scales by isd[dst] and writes the result slab.
  * Host: inverse-permute per-core outputs into global row order, + b2.
"""

import numpy as np

import concourse.bacc as bacc
import concourse.bass as bass
import concourse.mybir as mybir
from concourse.bass_utils import run_bass_kernel_spmd

# ---------------- problem constants (hardcoded per contract) ----------------
N = 50000          # nodes
FIN = 128          # input channels
FOUT = 64          # output channels
NCORES = 8
PER = N // NCORES  # 6250 dst nodes per core
TILES = 49         # ceil(PER/128)
PADN = TILES * 128  # 6272 padded dst slots per core

CHUNK = 4096       # xtm slots per DMA chunk (1 MB bf16)
NBANK = 8          # psum banks (512 f32 each) rotated across batches

_cache = {}

BF16 = mybir.dt.np(mybir.dt.bfloat16)


# ------------------------------ host schedule -------------------------------
def _build_schedule(src, dst):
    """Returns (isd, batches, percore, outmaps).

    batches: [(t0, g, db)] shared across cores (g in {1,2,4}, g*db % 4 == 0,
    so every degree layer of g*128 slots lies inside one 4096-slot chunk and
    every batch starts at a 512-slot boundary).
    percore[k]: dict(cols=int32[S] global source row per xtm column (N = zero
    row), isdb=[64, PADN] f32). outmaps[k]: slot -> global node id (-1 pad).
    """
    msrc = np.concatenate([src, np.arange(N, dtype=np.int64)])
    mdst = np.concatenate([dst, np.arange(N, dtype=np.int64)])
    deg = np.bincount(mdst, minlength=N)
    isd = (1.0 / np.sqrt(np.maximum(deg, 1))).astype(np.float32)

    core = mdst // PER
    dloc = mdst - core * PER

    pc = []
    Dmax = np.zeros(TILES, dtype=np.int64)
    for k in range(NCORES):
        sel = core == k
        d = dloc[sel]
        s = msrc[sel]
        cnt = np.bincount(d, minlength=PER)
        order = np.argsort(cnt, kind="stable")  # ascending degree
        pos = np.empty(PER, dtype=np.int64)
        pos[order] = np.arange(PER) + (PADN - PER)  # dummies at slots 0..21
        o2 = np.argsort(d, kind="stable")
        ds = d[o2]
        starts = np.searchsorted(ds, np.arange(PER))
        j = np.arange(ds.shape[0]) - starts[ds]  # rank within dst
        mpos = pos[ds]
        cntpad = np.zeros(PADN, dtype=np.int64)
        cntpad[pos] = cnt
        Dmax = np.maximum(Dmax, cntpad.reshape(TILES, 128).max(axis=1))
        pc.append(dict(pos=pos, tile=mpos // 128, pslot=mpos % 128,
                       j=j, src=s[o2]))

    # batches: consecutive tiles, g in {1,2,4}, db padded so g*db % 4 == 0
    batches = []
    t0 = 0
    while t0 < TILES:
        g = 1
        db = int(Dmax[t0])
        for gtry in (2, 4):
            if t0 + gtry > TILES:
                break
            nd = int(Dmax[t0:t0 + gtry].max())
            waste = nd * gtry - int(Dmax[t0:t0 + gtry].sum())
            if waste > max(2 * gtry, (nd * gtry) // 16):
                break
            g, db = gtry, nd
        while (g * db) % 4:
            db += 1
        batches.append((t0, g, db))
        t0 += g

    off = np.zeros(TILES, dtype=np.int64)   # xtm column offset of tile's batch
    t0of = np.zeros(TILES, dtype=np.int64)  # batch t0 of each tile
    gof = np.zeros(TILES, dtype=np.int64)   # batch g of each tile
    S = 0
    for (t0, g, db) in batches:
        off[t0:t0 + g] = S
        t0of[t0:t0 + g] = t0
        gof[t0:t0 + g] = g
        S += 128 * g * db

    percore = []
    outmaps = []
    for k in range(NCORES):
        e = pc[k]
        cols = np.full(S, N, dtype=np.int64)  # default: zero row
        t = e["tile"]
        lin = off[t] + e["j"] * (gof[t] * 128) + (t - t0of[t]) * 128 + e["pslot"]
        cols[lin] = e["src"]
        isdb = np.zeros(PADN, dtype=np.float32)
        isdb[e["pos"]] = isd[k * PER:(k + 1) * PER]
        isdb64 = np.ascontiguousarray(
            np.broadcast_to(isdb[None, :], (FOUT, PADN)))
        percore.append(dict(cols=cols.astype(np.int32), isdb=isdb64))
        om = np.full(PADN, -1, dtype=np.int64)
        om[e["pos"]] = np.arange(k * PER, (k + 1) * PER)
        outmaps.append(om)

    return isd, batches, percore, outmaps


# ------------------------------ device program ------------------------------
def _build_program(batches, reps=1):
    nc = bacc.Bacc("TRN2", debug=False)
    f32 = mybir.dt.float32
    bf16 = mybir.dt.bfloat16

    S = sum(128 * g * db for (_, g, db) in batches)
    NB = len(batches)
    NCH = -(-S // CHUNK)  # chunks per rep

    xtm = nc.declare_dram_parameter("xtm", [FIN, S], bf16, isOutput=False)
    w = nc.declare_dram_parameter("w", [FIN, FOUT], bf16, isOutput=False)
    isdb = nc.declare_dram_parameter("isdb", [FOUT, PADN], f32, isOutput=False)
    out = nc.declare_dram_parameter("out", [FOUT, PADN], f32, isOutput=True)

    # window list (one matmul per degree layer; never crosses a chunk)
    wins = []  # (chunk, rhs_off_in_chunk, psum_off, cols, start, stop, batch)
    col = 0
    for b, (t0, g, db) in enumerate(batches):
        W = g * 128
        for d in range(db):
            c = col // CHUNK
            assert col // CHUNK == (col + W - 1) // CHUNK
            wins.append((c, col % CHUNK, (b % NBANK) * 512, W,
                         d == 0, d == db - 1, b))
            col += W
    NW = len(wins)
    assert col == S

    # per-batch cumulative matmul counts (rep-local)
    mm_after = [0] * NB
    for wi, (_, _, _, _, _, _, b) in enumerate(wins):
        mm_after[b] = wi + 1
    # last rep-local batch using each psum bank (for cross-rep reuse gating)
    lastuser = [max(b for b in range(NB) if b % NBANK == k)
                for k in range(min(NBANK, NB))]
    # last window index touching each chunk
    wlast = [0] * NCH
    for wi, (c, _, _, _, _, _, _) in enumerate(wins):
        wlast[c] = wi + 1

    # output written in two pieces so most of the tail hides under the
    # last batches; piece 0 covers batches [0, NBH), piece 1 the rest
    NBH = NB // 2
    tile_h = batches[NBH][0]  # first tile of piece 1

    NBUF = 4  # chunk buffer rotation depth

    from contextlib import ExitStack
    with ExitStack() as ctx:
        w_sb = ctx.enter_context(nc.sbuf_tensor("w_sb", [FIN, FOUT], bf16))
        xtm_sb = ctx.enter_context(
            nc.sbuf_tensor("xtm_sb", [FIN, NBUF, CHUNK], bf16))
        isdb_sb = ctx.enter_context(
            nc.sbuf_tensor("isdb_sb", [FOUT, PADN], f32))
        resf = ctx.enter_context(nc.sbuf_tensor("resf", [FOUT, PADN], f32))
        ps = ctx.enter_context(nc.psum_tensor("ps", [FOUT, NBANK * 512], f32))
        names = ["PRMW", "PRMI", "MMC", "VCH", "BWOUT"]
        sem = {n: ctx.enter_context(nc.semaphore(n)) for n in names}
        PRMW, PRMI, MMC, VCH, BWOUT = (sem[n] for n in names)
        # per-buffer chunk-load sems: DMA completions are NOT ordered across
        # transfers (the small tail chunk finishes before its 1MB
        # predecessor), so one shared counter would let the PE stream a
        # buffer whose load is still in flight
        XTL = [ctx.enter_context(nc.semaphore("XTL%d" % i))
               for i in range(NBUF)]
        block = ctx.enter_context(nc.Block())

        @block.sync
        def _(s: bass.BassEngine):
            first = [True]

            def load(c, r):
                gc = r * NCH + c
                if gc >= NBUF:
                    pr, pcc = divmod(gc - NBUF, NCH)
                    s.wait_ge(MMC, pr * NW + wlast[pcc])
                n = min(CHUNK, S - c * CHUNK)
                s.dma_start(
                    xtm_sb[:, gc % NBUF, :n],
                    xtm[:, c * CHUNK: c * CHUNK + n],
                ).then_inc(XTL[gc % NBUF], 16)

            for r in range(reps):
                for c in range(NCH):
                    load(c, r)
                    if first[0]:
                        # params slotted behind the first chunks so the
                        # tensor engine can start as early as possible
                        if c == 0:
                            s.dma_start(w_sb[:], w[:]).then_inc(PRMW, 16)
                        elif c == 1:
                            s.dma_start(isdb_sb[:], isdb[:]).then_inc(PRMI, 16)
                            first[0] = False
                s.wait_ge(VCH, (r + 1) * NB)
                s.dma_start(out[:], resf[:]).then_inc(BWOUT, 16)
                # full inter-rep fence: BWOUT transitively proves all of
                # this rep's work (XTL -> MMC -> VCH -> out) completed
                s.wait_ge(BWOUT, 16 * (r + 1))

        @block.tensor
        def _(t: bass.BassTensorEngine):
            t.wait_ge(PRMW, 16)
            for r in range(reps):
                if r > 0:
                    t.wait_ge(BWOUT, 16 * r)  # inter-rep fence
                for wi, (c, co, po, W, st, sp, b) in enumerate(wins):
                    gb = r * NB + b
                    gcw = r * NCH + c
                    if st:
                        # psum bank b%NBANK reuse: wait for its previous
                        # user's DVE read (rep-local bank assignment)
                        if b >= NBANK:
                            t.wait_ge(VCH, r * NB + (b - NBANK) + 1)
                        elif r > 0:
                            t.wait_ge(VCH,
                                      (r - 1) * NB + lastuser[b % NBANK] + 1)
                        t.wait_ge(XTL[gcw % NBUF], 16 * (gcw // NBUF + 1))
                    elif co == 0:
                        # first window of a new chunk mid-batch
                        t.wait_ge(XTL[gcw % NBUF], 16 * (gcw // NBUF + 1))
                    t.matmul(
                        out=ps[:, po: po + W],
                        lhsT=w_sb[:],
                        rhs=xtm_sb[:, (r * NCH + c) % NBUF, co: co + W],
                        start=st, stop=sp,
                    ).then_inc(MMC, 1)

        @block.vector
        def _(v: bass.BassVectorEngine):
            v.wait_ge(PRMI, 16)
            for r in range(reps):
                for b, (t0, g, db) in enumerate(batches):
                    gb = r * NB + b
                    v.wait_ge(MMC, r * NW + mm_after[b])
                    if r > 0 and b == 0:
                        # resf drained by the previous repeat's write
                        v.wait_ge(BWOUT, 16 * r)
                    v.tensor_tensor(
                        out=resf[:, t0 * 128: (t0 + g) * 128],
                        in0=ps[:, (b % NBANK) * 512: (b % NBANK) * 512 + g * 128],
                        in1=isdb_sb[:, t0 * 128: (t0 + g) * 128],
                        op=mybir.AluOpType.mult,
                    )
                    # sem inc via drain: a DVE op's own then_inc can fire
                    # before its SBUF writes are visible to other engines
                    v.drain().then_inc(VCH, 1)

    nc.compile()
    return nc


# --------------------------------- kernel -----------------------------------
def prepare(edges, features, W2, b2):
    """Build (nc, in_maps, assemble) for the given full inputs."""
    edges = np.asarray(edges)
    X = np.asarray(features, dtype=np.float32)
    W2 = np.asarray(W2, dtype=np.float32)
    b2 = np.asarray(b2, dtype=np.float32)
    src = edges[0].astype(np.int64)
    dst = edges[1].astype(np.int64)

    isd, batches, percore, outmaps = _build_schedule(src, dst)

    key = tuple(batches)
    if key not in _cache:
        _cache[key] = _build_program(batches)
    nc = _cache[key]

    # XpT: [128, N+1] bf16, col n = isd[n] * X[n]; col N is zero
    XpT = np.zeros((FIN, N + 1), dtype=BF16)
    XpT[:, :N] = (X * isd[:, None]).T.astype(BF16)
    Wb = W2.astype(BF16)

    in_maps = []
    for k in range(NCORES):
        in_maps.append(dict(
            xtm=np.ascontiguousarray(XpT[:, percore[k]["cols"]]),
            w=Wb,
            isdb=percore[k]["isdb"],
        ))

    def assemble(results):
        z = np.empty((N, FOUT), dtype=np.float32)
        for k in range(NCORES):
            om = outmaps[k]
            valid = om >= 0
            z[om[valid]] = results[k]["out"].T[valid]
        return z + b2[None, :]

    return nc, in_maps, assemble


def kernel(edges, features, W1, b1, W2, b2):
    nc, in_maps, assemble = prepare(edges, features, W2, b2)
    res = run_bass_kernel_spmd(nc, in_maps, list(range(NCORES)))
    return assemble(res.results)


# revision 23
# speedup vs baseline: 1.7730x; 1.7730x over previous
"""GCN (PyG GCNConv) forward on 8 Trainium2 NeuronCores.

Reference computes z = D^-1/2 (A+I) D^-1/2 (X @ W2) + b2  (conv1 is dead code,
its result is never used).

Strategy ("message-GEMM", 1D destination partition):
  * Host: compute degrees + symmetric normalization, fold isd[src] into X
    (bf16), partition messages (edges + self loops) by destination shard
    (8 cores x 6250 nodes), degree-sort dst slots, and materialize the
    per-message source columns as one dense bf16 operand
    xtm[128 feats, S slots] per core (padding columns are zero).
  * Device (identical program on all 8 cores, per-core data via in_maps):
    stream xtm sequentially (full HBM rate, no gather descriptors at all);
    for each batch of g dst tiles the degree layers are accumulated directly
    in PSUM by the tensor engine:
        psum[64, g*128] (+)= W2.T @ xtm[:, layer d columns]   d = 0..db-1
    so the segmented sum over incoming messages IS the matmul accumulation.
    One DVE op per batch scales by isd[dst] and writes the result slab.
  * Host: inverse-permute per-core outputs into global row order, + b2.
"""

import numpy as np

import concourse.bacc as bacc
import concourse.bass as bass
import concourse.mybir as mybir
from concourse.bass_utils import run_bass_kernel_spmd

# ---------------- problem constants (hardcoded per contract) ----------------
N = 50000          # nodes
FIN = 128          # input channels
FOUT = 64          # output channels
NCORES = 8
PER = N // NCORES  # 6250 dst nodes per core
TILES = 49         # ceil(PER/128)
PADN = TILES * 128  # 6272 padded dst slots per core

CHUNK = 4096       # xtm slots per DMA chunk (1 MB bf16)
NBANK = 8          # psum banks (512 f32 each) rotated across batches

_cache = {}

BF16 = mybir.dt.np(mybir.dt.bfloat16)


# ------------------------------ host schedule -------------------------------
def _build_schedule(src, dst):
    """Returns (isd, batches, percore, outmaps).

    batches: [(t0, g, db)] shared across cores (g in {1,2,4}, g*db % 4 == 0,
    so every degree layer of g*128 slots lies inside one 4096-slot chunk and
    every batch starts at a 512-slot boundary).
    percore[k]: dict(cols=int32[S] global source row per xtm column (N = zero
    row), isdb=[64, PADN] f32). outmaps[k]: slot -> global node id (-1 pad).
    """
    msrc = np.concatenate([src, np.arange(N, dtype=np.int64)])
    mdst = np.concatenate([dst, np.arange(N, dtype=np.int64)])
    deg = np.bincount(mdst, minlength=N)
    isd = (1.0 / np.sqrt(np.maximum(deg, 1))).astype(np.float32)

    core = mdst // PER
    dloc = mdst - core * PER

    pc = []
    Dmax = np.zeros(TILES, dtype=np.int64)
    for k in range(NCORES):
        sel = core == k
        d = dloc[sel]
        s = msrc[sel]
        cnt = np.bincount(d, minlength=PER)
        order = np.argsort(cnt, kind="stable")  # ascending degree
        pos = np.empty(PER, dtype=np.int64)
        pos[order] = np.arange(PER) + (PADN - PER)  # dummies at slots 0..21
        o2 = np.argsort(d, kind="stable")
        ds = d[o2]
        starts = np.searchsorted(ds, np.arange(PER))
        j = np.arange(ds.shape[0]) - starts[ds]  # rank within dst
        mpos = pos[ds]
        cntpad = np.zeros(PADN, dtype=np.int64)
        cntpad[pos] = cnt
        Dmax = np.maximum(Dmax, cntpad.reshape(TILES, 128).max(axis=1))
        pc.append(dict(pos=pos, tile=mpos // 128, pslot=mpos % 128,
                       j=j, src=s[o2]))

    # batches: consecutive tiles, g in {1,2,4}, db padded so g*db % 4 == 0
    batches = []
    t0 = 0
    while t0 < TILES:
        g = 1
        db = int(Dmax[t0])
        for gtry in (2, 4):
            if t0 + gtry > TILES:
                break
            nd = int(Dmax[t0:t0 + gtry].max())
            waste = nd * gtry - int(Dmax[t0:t0 + gtry].sum())
            if waste > max(2 * gtry, (nd * gtry) // 16):
                break
            g, db = gtry, nd
        while (g * db) % 4:
            db += 1
        batches.append((t0, g, db))
        t0 += g

    off = np.zeros(TILES, dtype=np.int64)   # xtm column offset of tile's batch
    t0of = np.zeros(TILES, dtype=np.int64)  # batch t0 of each tile
    gof = np.zeros(TILES, dtype=np.int64)   # batch g of each tile
    S = 0
    for (t0, g, db) in batches:
        off[t0:t0 + g] = S
        t0of[t0:t0 + g] = t0
        gof[t0:t0 + g] = g
        S += 128 * g * db

    percore = []
    outmaps = []
    for k in range(NCORES):
        e = pc[k]
        cols = np.full(S, N, dtype=np.int64)  # default: zero row
        t = e["tile"]
        lin = off[t] + e["j"] * (gof[t] * 128) + (t - t0of[t]) * 128 + e["pslot"]
        cols[lin] = e["src"]
        isdb = np.zeros(PADN, dtype=np.float32)
        isdb[e["pos"]] = isd[k * PER:(k + 1) * PER]
        isdb64 = np.ascontiguousarray(
            np.broadcast_to(isdb[None, :], (FOUT, PADN)))
        percore.append(dict(cols=cols.astype(np.int32), isdb=isdb64))
        om = np.full(PADN, -1, dtype=np.int64)
        om[e["pos"]] = np.arange(k * PER, (k + 1) * PER)
        outmaps.append(om)

    return isd, batches, percore, outmaps


# ------------------------------ device program ------------------------------
def _build_program(batches, reps=1):
    nc = bacc.Bacc("TRN2", debug=False)
    f32 = mybir.dt.float32
    bf16 = mybir.dt.bfloat16

    S = sum(128 * g * db for (_, g, db) in batches)
    NB = len(batches)
    NCH = -(-S // CHUNK)  # chunks per rep

    xtm = nc.declare_dram_parameter("xtm", [FIN, S], bf16, isOutput=False)
    w = nc.declare_dram_parameter("w", [FIN, FOUT], bf16, isOutput=False)
    isdb = nc.declare_dram_parameter("isdb", [FOUT, PADN], f32, isOutput=False)
    out = nc.declare_dram_parameter("out", [FOUT, PADN], f32, isOutput=True)

    # window list (one matmul per degree layer; never crosses a chunk)
    wins = []  # (chunk, rhs_off_in_chunk, psum_off, cols, start, stop, batch)
    col = 0
    for b, (t0, g, db) in enumerate(batches):
        W = g * 128
        for d in range(db):
            c = col // CHUNK
            assert col // CHUNK == (col + W - 1) // CHUNK
            wins.append((c, col % CHUNK, (b % NBANK) * 512, W,
                         d == 0, d == db - 1, b))
            col += W
    NW = len(wins)
    assert col == S

    # per-batch cumulative matmul counts (rep-local)
    mm_after = [0] * NB
    for wi, (_, _, _, _, _, _, b) in enumerate(wins):
        mm_after[b] = wi + 1
    # last rep-local batch using each psum bank (for cross-rep reuse gating)
    lastuser = [max(b for b in range(NB) if b % NBANK == k)
                for k in range(min(NBANK, NB))]
    # last window index touching each chunk
    wlast = [0] * NCH
    for wi, (c, _, _, _, _, _, _) in enumerate(wins):
        wlast[c] = wi + 1

    NBUF = 4  # chunk buffer rotation depth

    from contextlib import ExitStack
    with ExitStack() as ctx:
        w_sb = ctx.enter_context(nc.sbuf_tensor("w_sb", [FIN, FOUT], bf16))
        xtm_sb = ctx.enter_context(
            nc.sbuf_tensor("xtm_sb", [FIN, NBUF, CHUNK], bf16))
        isdb_sb = ctx.enter_context(
            nc.sbuf_tensor("isdb_sb", [FOUT, PADN], f32))
        resf = ctx.enter_context(nc.sbuf_tensor("resf", [FOUT, PADN], f32))
        ps = ctx.enter_context(nc.psum_tensor("ps", [FOUT, NBANK * 512], f32))
        names = ["PRMW", "PRMI", "MMC", "VCH", "BWOUT"]
        sem = {n: ctx.enter_context(nc.semaphore(n)) for n in names}
        PRMW, PRMI, MMC, VCH, BWOUT = (sem[n] for n in names)
        # per-buffer chunk-load sems: DMA completions are NOT ordered across
        # transfers (the small tail chunk finishes before its 1MB
        # predecessor), so one shared counter would let the PE stream a
        # buffer whose load is still in flight
        XTL = [ctx.enter_context(nc.semaphore("XTL%d" % i))
               for i in range(NBUF)]
        block = ctx.enter_context(nc.Block())

        @block.sync
        def _(s: bass.BassEngine):
            first = [True]

            def load(c, r):
                gc = r * NCH + c
                if gc >= NBUF:
                    pr, pcc = divmod(gc - NBUF, NCH)
                    s.wait_ge(MMC, pr * NW + wlast[pcc])
                n = min(CHUNK, S - c * CHUNK)
                s.dma_start(
                    xtm_sb[:, gc % NBUF, :n],
                    xtm[:, c * CHUNK: c * CHUNK + n],
                ).then_inc(XTL[gc % NBUF], 16)

            for r in range(reps):
                for c in range(NCH):
                    load(c, r)
                    if first[0]:
                        # params slotted behind the first chunks so the
                        # tensor engine can start as early as possible
                        if c == 0:
                            s.dma_start(w_sb[:], w[:]).then_inc(PRMW, 16)
                        elif c == 1:
                            s.dma_start(isdb_sb[:], isdb[:]).then_inc(PRMI, 16)
                            first[0] = False
                s.wait_ge(VCH, (r + 1) * NB)
                s.dma_start(out[:], resf[:]).then_inc(BWOUT, 16)
                # full inter-rep fence: BWOUT transitively proves all of
                # this rep's work (XTL -> MMC -> VCH -> out) completed
                s.wait_ge(BWOUT, 16 * (r + 1))

        @block.tensor
        def _(t: bass.BassTensorEngine):
            t.wait_ge(PRMW, 16)
            for r in range(reps):
                if r > 0:
                    t.wait_ge(BWOUT, 16 * r)  # inter-rep fence
                cur_chunk = -1
                prev_b = -1
                for wi, (c, co, po, W, st, sp, b) in enumerate(wins):
                    gcw = r * NCH + c
                    if b != prev_b:
                        # psum bank b%NBANK reuse: wait for its previous
                        # user's DVE read (rep-local bank assignment)
                        if b >= NBANK:
                            t.wait_ge(VCH, r * NB + (b - NBANK) + 1)
                        elif r > 0:
                            t.wait_ge(VCH,
                                      (r - 1) * NB + lastuser[b % NBANK] + 1)
                        prev_b = b
                    if c != cur_chunk:
                        t.wait_ge(XTL[gcw % NBUF], 16 * (gcw // NBUF + 1))
                        cur_chunk = c
                    t.matmul(
                        out=ps[:, po: po + W],
                        lhsT=w_sb[:],
                        rhs=xtm_sb[:, gcw % NBUF, co: co + W],
                        start=st, stop=sp,
                    ).then_inc(MMC, 1)

        @block.vector
        def _(v: bass.BassVectorEngine):
            v.wait_ge(PRMI, 16)
            for r in range(reps):
                for b, (t0, g, db) in enumerate(batches):
                    v.wait_ge(MMC, r * NW + mm_after[b])
                    if r > 0 and b == 0:
                        # resf drained by the previous repeat's write
                        v.wait_ge(BWOUT, 16 * r)
                    v.tensor_tensor(
                        out=resf[:, t0 * 128: (t0 + g) * 128],
                        in0=ps[:, (b % NBANK) * 512: (b % NBANK) * 512 + g * 128],
                        in1=isdb_sb[:, t0 * 128: (t0 + g) * 128],
                        op=mybir.AluOpType.mult,
                    )
                    # sem inc via drain: a DVE op's own then_inc can fire
                    # before its SBUF writes are visible to other engines
                    v.drain().then_inc(VCH, 1)

    nc.compile()
    return nc


# --------------------------------- kernel -----------------------------------
def prepare(edges, features, W2, b2):
    """Build (nc, in_maps, assemble) for the given full inputs."""
    edges = np.asarray(edges)
    X = np.asarray(features, dtype=np.float32)
    W2 = np.asarray(W2, dtype=np.float32)
    b2 = np.asarray(b2, dtype=np.float32)
    src = edges[0].astype(np.int64)
    dst = edges[1].astype(np.int64)

    isd, batches, percore, outmaps = _build_schedule(src, dst)

    key = tuple(batches)
    if key not in _cache:
        _cache[key] = _build_program(batches)
    nc = _cache[key]

    # XpT: [128, N+1] bf16, col n = isd[n] * X[n]; col N is zero
    XpT = np.zeros((FIN, N + 1), dtype=BF16)
    XpT[:, :N] = (X * isd[:, None]).T.astype(BF16)
    Wb = W2.astype(BF16)

    in_maps = []
    for k in range(NCORES):
        in_maps.append(dict(
            xtm=np.ascontiguousarray(XpT[:, percore[k]["cols"]]),
            w=Wb,
            isdb=percore[k]["isdb"],
        ))

    def assemble(results):
        z = np.empty((N, FOUT), dtype=np.float32)
        for k in range(NCORES):
            om = outmaps[k]
            valid = om >= 0
            z[om[valid]] = results[k]["out"].T[valid]
        return z + b2[None, :]

    return nc, in_maps, assemble


def kernel(edges, features, W1, b1, W2, b2):
    nc, in_maps, assemble = prepare(edges, features, W2, b2)
    res = run_bass_kernel_spmd(nc, in_maps, list(range(NCORES)))
    return assemble(res.results)
